# revision 32
# baseline (speedup 1.0000x reference)
"""Trainium2 Bass kernel for single-head attention (no V projection).

Reference computation (per batch b):
    q = x @ Wq ; k = x @ Wk
    scores = q @ k.T / sqrt(64)
    out = softmax(scores, axis=-1) @ x

Shapes: x [4, 2048, 1024], Wq/Wk [1024, 1024] -> out [4, 2048, 1024] fp32.

Key algebraic reduction: with no V projection, scores = x (Wq Wk^T / 8)
x^T, so the host precomputes M = Wq @ Wk.T / 8 (one 1024^3 sgemm in
numpy) and the device never projects k at all.  Each core projects only
its own 1024 query rows (y = x_q @ M, 27us of PE) instead of q plus the
full 2048-row k (82us), which also removes the duplicated-k work the
query-split sharding would otherwise pay.  Per-core PE work: y-proj
27us + scores 55us + attn@x 55us ~= 137us.

Sharding: 8 cores, core c handles batch b=c//2, query-row half h=c%2.
Each core receives its batch's x rolled so its 1024 query rows come
first (attention is permutation-invariant over keys), plus the same x
pre-transposed on the host (xt) - the PE contracts over the partition
dim, so the y projection and the scores lhsT need x with the model dim
on partitions, and trn2 has no fp32 DMA-transpose while PE-transpose
costs ~300ns per 128x128 tile.  No collectives.

On-chip dataflow (all matmuls contract over the partition dim):
    yT  [d, s]  = M.T @ x_q.T  (lhsT=M tile, rhs=xt[:, 0:1024])
    scoresT [t, s] = xT.T-chunks @ yT   (lhsT=xT, rhs=yT)
    expT = Exp(scoresT)        (ScalarE eviction from PSUM)
    sumexp [s, 2] = expT.T @ ones      (N=2 matmuls; fp32r needs N>=2)
    out [s, d] = expT.T @ x            (lhsT=expT, rhs=x natural)
    out scaled by 1/sumexp on the DVE during PSUM->SBUF eviction.

A burst of warmup matmuls on uninitialized SBUF (no DMA dependency)
runs from ~6.8us so the PE's HAM clock-gate reaches full speed and the
PE never idles >3.4us (which would re-gate it) before the first real
matmuls issue at ~13us when their inputs land.  The first ~15us of
real (random-data) matmuls then run at 35-50% rate while the chip's
DVFS brings the rail up - all 8 cores ramp simultaneously; measured
across many variants this bring-up cost is conserved (~10-12us) no
matter how the warmup or work onset is arranged.

Matmul operands live in SBUF as float32r (fp32 bits; the PE truncates
to FP22 on read - 1 cycle/row at free-dim>=256 vs 4 for fp32; measured
end-to-end rel err ~6e-4). The BIR verifier requires every producer of
an fp32r matmul operand to write fp32r-typed data, so DMA'd tiles use
fp32r DRAM params and PSUM evictions write to fp32r tiles.

Softmax skips the max-subtraction: scores have std ~4 and |max| < ~25,
so exp stays comfortably inside fp32 range and the result is
mathematically identical to jax.nn.softmax.
"""

from contextlib import ExitStack

import numpy as np

import concourse.bacc as bacc
import concourse.tile as tile
from concourse import mybir
from concourse.bass_utils import run_bass_kernel_spmd

F32 = mybir.dt.float32
F32R = mybir.dt.float32r
BF16 = mybir.dt.bfloat16
AFT = mybir.ActivationFunctionType

P = 128      # partitions
S = 2048     # keys (t) per batch
SQ = 1024    # query rows per core
D = 1024     # model dim
NT = S // P  # 16 t-chunks
ND = D // P  # 8 d-chunks
SB = 512     # query-block width in phase C
NSB = SQ // SB
# Warmup: matmuls on uninitialized (zero) SBUF - no DMA dependency, so
# they start at ~6.8us and release the PE HAM clock-gate (1.2->2.4GHz
# ramp), sized to end right as the first y-proj inputs land (~12.5us).
# Zero operands draw almost no multiplier power, so they run at full
# rate; the DVFS activity throttle then trips ~1.5us into the real
# (random-data) matmuls and caps util for ~15us wall.  Measured: a
# full-rate random-data warmup burst (256-wide on m0) pre-triggered the
# throttle but made the recovery LONGER (20.2us vs 14.9us from power
# onset) - net wash - so the warmup stays on zeros and the real work
# just eats the bring-up window at ~35-50% rate.
N_WARMUP_A = 16

B_FULL, S_FULL, D_FULL = 4, 2048, 1024
N_CORES = 8

_NC_CACHE = None
LAST_RESULT = None  # BassKernelResults of the most recent kernel() call
TRACE = False      # set by test.py to capture an NTFF profile
TRACE_DIR = None


def _r(ap):
    return ap.bitcast(F32R)


def _build_nc():
    global _NC_CACHE
    if _NC_CACHE is not None:
        return _NC_CACHE

    nc = bacc.Bacc("TRN2")
    x = nc.declare_dram_parameter("x", [S, D], F32R, isOutput=False)
    xt = nc.declare_dram_parameter("xt", [D, S], F32R, isOutput=False)
    # m pre-gathered on the host into e-major SBUF layout
    # (mg[p, e*1024 + dd*128 + j] = m[dd*128 + p, e*128 + j]): y-proj
    # group e needs m's column block e across all dd row-chunks, and as
    # strided slices of a row-major m those stream at 512-byte packets
    # (measured: half-rate DMA that starved the y projection for ~4us);
    # contiguous per-e blocks move in single [128 x 4KB] full-rate DMAs
    # issued in exact need order.
    mg = nc.declare_dram_parameter("mg", [P, ND * D], F32R, isOutput=False)
    ones = nc.declare_dram_parameter("ones", [P, 2], F32R, isOutput=False)
    out = nc.declare_dram_parameter("out", [SQ, D], F32, isOutput=True)

    with tile.TileContext(nc) as tc, ExitStack() as ctx:
        singles = ctx.enter_context(tc.tile_pool(name="singles", bufs=1))
        ot = singles.tile([P, 2], F32R)

        persist = ctx.enter_context(tc.tile_pool(name="persist", bufs=1))
        # d-chunk d at [:, d*SQ : (d+1)*SQ] (free axis = s)
        yT = persist.tile([P, ND * SQ], F32R, tag="yT")
        # d-chunk d at [:, d*S : (d+1)*S] (free axis = s/t); query rows
        # first (s<1024), key-only rows after.  Persists into phase C as
        # the scores lhsT.
        xT = persist.tile([P, ND * S], F32R, tag="xT")
        # Shared matmul-accumulator pool for the y projection AND the
        # scores groups: keeping one rotation means the first scores
        # group lands in the least-recently-evicted bank instead of
        # waiting on the final y-proj eviction (measured 846ns stall).
        ps_main = ctx.enter_context(
            tc.tile_pool(name="ps_main", bufs=4, space="PSUM"))

        # ---------- phase B: load xt/m, project y ----------
        with tc.tile_pool(name="wA", bufs=1) as wA_pool, \
             tc.tile_pool(name="ps_w", bufs=1, space="PSUM") as ps_w_pool:
            # e-block e at [:, e*D : (e+1)*D]; lhsT for (dd, e) at
            # [:, e*D + dd*P : e*D + (dd+1)*P]
            wt = wA_pool.tile([P, ND * D], F32R, tag="m")

            # Warmup A: clock-ramp matmuls on uninitialized yT contents
            # (yT's first real write is a DVE eviction much later, so no
            # dependency and no delay); garbage results land in a scratch
            # PSUM bank that is never read.
            ps_w = ps_w_pool.tile([P, 512], F32)
            for i in range(N_WARMUP_A):
                nc.tensor.matmul(ps_w[:], yT[:, 0:P], yT[:, SQ:SQ + 512],
                                 start=(i == 0), stop=(i == N_WARMUP_A - 1))

            # DMA order drives arrival order, and through the DVFS window
            # DMA supply and (throttled) PE demand are both marginal, so
            # stream in EXACT need order: m's e=0 block, the xt query
            # stripes, then m's remaining e-blocks.  (Interleaving mg0's
            # dd-chunks with the xt stripes to start the first matmul on
            # 320KB was tried and lost 4us: real DMA packets only start
            # flowing ~8.3us in, the PE went idle >3.4us and re-gated.)
            nc.sync.dma_start(out=wt[:, 0:D], in_=mg[:, 0:D])
            for sh in range(SQ // 512):
                for d in range(ND):
                    nc.sync.dma_start(
                        out=xT[:, d * S + sh * 512: d * S + sh * 512 + 512],
                        in_=xt[d * P:(d + 1) * P, sh * 512: sh * 512 + 512],
                    )
            for e in range(1, ND):
                nc.sync.dma_start(
                    out=wt[:, e * D:(e + 1) * D], in_=mg[:, e * D:(e + 1) * D]
                )
            # xT key-only half (s 1024..2048): needed from scores t=8 on.
            for dd in range(ND):
                nc.sync.dma_start(
                    out=xT[:, dd * S + SQ: dd * S + S],
                    in_=xt[dd * P:(dd + 1) * P, SQ:S],
                )
            # ones (for the sumexp partition-reduce, first used ~84us)
            # issued dead last among phase-B DMAs: at the head its
            # descriptor slot (~0.6us on the Sync queue) would delay mg0.
            nc.sync.dma_start(out=ot[:], in_=ones[:])

            # yT projection; 512-wide stripes only - narrower ones are
            # LDWEIGHTS-bound (190ns load > 107ns mm)
            for e in range(ND):
                for sh in range(SQ // 512):
                    ps = ps_main.tile([P, 512], F32)
                    for dd in range(ND):
                        lhs = wt[:, e * D + dd * P: e * D + (dd + 1) * P]
                        nc.tensor.matmul(
                            ps[:],
                            lhs,
                            xT[:, dd * S + sh * 512: dd * S + sh * 512 + 512],
                            start=(dd == 0), stop=(dd == ND - 1),
                        )
                    nc.vector.tensor_copy(
                        yT[:, e * SQ + sh * 512: e * SQ + sh * 512 + 512],
                        _r(ps[:]),
                    )

        # ---------- phase C: scores -> softmax -> attn @ x ----------
        with tc.tile_pool(name="xc", bufs=1) as xc_pool, \
             tc.tile_pool(name="exp", bufs=1) as exp_pool, \
             tc.tile_pool(name="outp", bufs=4) as out_pool, \
             tc.tile_pool(name="recip", bufs=4) as recip_pool, \
             tc.tile_pool(name="partial", bufs=2) as partial_pool, \
             tc.tile_pool(name="ps_av", bufs=3, space="PSUM") as ps_av, \
             tc.tile_pool(name="ps_sum", bufs=1, space="PSUM") as ps_sum:
            # x natural: t-chunk t at [:, t*D : (t+1)*D]
            xc = xc_pool.tile([P, NT * D], F32R)
            for t in range(NT):
                nc.sync.dma_start(
                    out=xc[:, t * D:(t + 1) * D], in_=x[t * P:(t + 1) * P, :]
                )

            for blk in range(NSB):
                # t-chunk t at [:, t*SB : (t+1)*SB] (free axis = s within blk)
                # (expT must stay f32r: the BIR verifier requires both matmul
                # operands to match when either is f32/f32r, so bf16 expT
                # would force bf16 xc too; and the steady 227ns/mm interval is
                # mm(213) + ~14ns fixed pipeline handoff with LDWEIGHTS fully
                # parallel - faster bf16 weight loads would not shrink it.)
                expT = exp_pool.tile([P, NT * SB], F32R, tag="expT")
                # Softmax denominator: the 16-chunk accumulation runs on
                # the (otherwise idle) DVE as a chain of adds interleaved
                # with the scores loop; the PE then only does one N=2
                # partition-reduce matmul per s-chunk instead of 16
                # LDWEIGHTS-bound ones each (~115ns apiece, ~18us total).
                partial = partial_pool.tile([P, SB], F32R, tag="partial")
                for t in range(NT):
                    ps = ps_main.tile([P, SB], F32)
                    for e in range(ND):
                        nc.tensor.matmul(
                            ps[:],
                            xT[:, e * S + t * P: e * S + (t + 1) * P],
                            yT[:, e * SQ + blk * SB: e * SQ + (blk + 1) * SB],
                            start=(e == 0), stop=(e == ND - 1),
                        )
                    nc.scalar.activation(expT[:, t * SB:(t + 1) * SB], ps[:], AFT.Exp)
                    if t == 1:
                        nc.vector.tensor_add(
                            partial[:], expT[:, 0:SB], expT[:, SB:2 * SB])
                    elif t >= 2:
                        nc.vector.tensor_add(
                            partial[:], partial[:],
                            expT[:, t * SB:(t + 1) * SB])

                for ss in range(SB // P):
                    pss = ps_sum.tile([P, 2], F32)
                    nc.tensor.matmul(
                        pss[:], partial[:, ss * P:(ss + 1) * P], ot[:],
                        start=True, stop=True,
                    )
                    rec = recip_pool.tile([P, 1], F32, tag="rec")
                    nc.vector.reciprocal(rec[:], pss[:, 0:1])

                    for dh in range(2):
                        psa = ps_av.tile([P, 512], F32)
                        for t in range(NT):
                            nc.tensor.matmul(
                                psa[:],
                                expT[:, t * SB + ss * P: t * SB + (ss + 1) * P],
                                xc[:, t * D + dh * 512: t * D + dh * 512 + 512],
                                start=(t == 0), stop=(t == NT - 1),
                            )
                        ob = out_pool.tile([P, 512], F32, tag="ob")
                        row0 = blk * SB + ss * P
                        last = (blk == NSB - 1 and ss == SB // P - 1
                                and dh == 1)
                        if last:
                            # Final block: evict + DMA in two halves, one on
                            # the DVE and one on the ScalarE (Copy with
                            # per-partition scale) so both run concurrently -
                            # this eviction+DMA is the exposed tail after the
                            # last matmul.
                            nc.scalar.activation(
                                ob[:, 256:512], psa[:, 256:512], AFT.Copy,
                                scale=rec[:, 0:1])
                            nc.sync.dma_start(
                                out=out[row0:row0 + P,
                                        dh * 512 + 256:dh * 512 + 512],
                                in_=ob[:, 256:512],
                            )
                            nc.vector.tensor_scalar_mul(
                                ob[:, 0:256], psa[:, 0:256], rec[:, 0:1])
                            nc.sync.dma_start(
                                out=out[row0:row0 + P,
                                        dh * 512:dh * 512 + 256],
                                in_=ob[:, 0:256],
                            )
                        else:
                            nc.vector.tensor_scalar_mul(ob[:], psa[:], rec[:, 0:1])
                            nc.sync.dma_start(
                                out=out[row0:row0 + P, dh * 512:dh * 512 + 512],
                                in_=ob[:],
                            )

    nc.finalize()
    _NC_CACHE = nc
    return nc


def kernel(inputs, Wq, Wk):
    global LAST_RESULT
    x = np.asarray(inputs, dtype=np.float32)
    assert x.shape == (B_FULL, S_FULL, D_FULL)
    # scores = x (Wq Wk^T / 8) x^T: fold the projections and the softmax
    # scale into one host-side sgemm.
    m = (np.asarray(Wq, dtype=np.float32) @ np.asarray(Wk, dtype=np.float32).T) \
        * np.float32(0.125)
    # e-major SBUF layout: mg[p, e*1024 + dd*128 + j] = m[dd*128+p, e*128+j]
    mg = np.ascontiguousarray(
        m.reshape(ND, P, ND, P).transpose(1, 2, 0, 3).reshape(P, ND * D))
    ones = np.ones((P, 2), dtype=np.float32)

    nc = _build_nc()

    in_maps = []
    for c in range(N_CORES):
        b, h = c // 2, c % 2
        xb = x[b]
        if h:
            xb = np.concatenate([xb[SQ:], xb[:SQ]], axis=0)
        in_maps.append({
            "x": np.ascontiguousarray(xb),
            "xt": np.ascontiguousarray(xb.T),
            "mg": mg,
            "ones": ones,
        })

    kwargs = {}
    if TRACE:
        kwargs = {"trace": True, "tmpdir": TRACE_DIR}
    res = run_bass_kernel_spmd(nc, in_maps, list(range(N_CORES)), **kwargs)
    LAST_RESULT = res

    full = np.empty((B_FULL, S_FULL, D_FULL), dtype=np.float32)
    for c in range(N_CORES):
        b, h = c // 2, c % 2
        full[b, h * SQ:(h + 1) * SQ, :] = res.results[c]["out"]
    return full


# revision 33
# speedup vs baseline: 1.0012x; 1.0012x over previous
"""Trainium2 Bass kernel for single-head attention (no V projection).

Reference computation (per batch b):
    q = x @ Wq ; k = x @ Wk
    scores = q @ k.T / sqrt(64)
    out = softmax(scores, axis=-1) @ x

Shapes: x [4, 2048, 1024], Wq/Wk [1024, 1024] -> out [4, 2048, 1024] fp32.

Key algebraic reduction: with no V projection, scores = x (Wq Wk^T / 8)
x^T, so the host precomputes M = Wq @ Wk.T / 8 (one 1024^3 sgemm in
numpy) and the device never projects k at all.  Each core projects only
its own 1024 query rows (y = x_q @ M, 27us of PE) instead of q plus the
full 2048-row k (82us), which also removes the duplicated-k work the
query-split sharding would otherwise pay.  Per-core PE work: y-proj
27us + scores 55us + attn@x 55us ~= 137us.

Sharding: 8 cores, core c handles batch b=c//2, query-row half h=c%2.
Each core receives its batch's x rolled so its 1024 query rows come
first (attention is permutation-invariant over keys), plus the same x
pre-transposed on the host (xt) - the PE contracts over the partition
dim, so the y projection and the scores lhsT need x with the model dim
on partitions, and trn2 has no fp32 DMA-transpose while PE-transpose
costs ~300ns per 128x128 tile.  No collectives.

On-chip dataflow (all matmuls contract over the partition dim):
    yT  [d, s]  = M.T @ x_q.T  (lhsT=M tile, rhs=xt[:, 0:1024])
    scoresT [t, s] = xT.T-chunks @ yT   (lhsT=xT, rhs=yT)
    expT = Exp(scoresT)        (ScalarE eviction from PSUM)
    sumexp [s, 2] = expT.T @ ones      (N=2 matmuls; fp32r needs N>=2)
    out [s, d] = expT.T @ x            (lhsT=expT, rhs=x natural)
    out scaled by 1/sumexp on the DVE during PSUM->SBUF eviction.

A burst of warmup matmuls on uninitialized SBUF (no DMA dependency)
runs from ~6.8us so the PE's HAM clock-gate reaches full speed and the
PE never idles >3.4us (which would re-gate it) before the first real
matmuls issue at ~13us when their inputs land.  The first ~15us of
real (random-data) matmuls then run at 35-50% rate while the chip's
DVFS brings the rail up - all 8 cores ramp simultaneously; measured
across many variants this bring-up cost is conserved (~10-12us) no
matter how the warmup or work onset is arranged.

Matmul operands live in SBUF as float32r (fp32 bits; the PE truncates
to FP22 on read - 1 cycle/row at free-dim>=256 vs 4 for fp32; measured
end-to-end rel err ~6e-4). The BIR verifier requires every producer of
an fp32r matmul operand to write fp32r-typed data, so DMA'd tiles use
fp32r DRAM params and PSUM evictions write to fp32r tiles.

Softmax skips the max-subtraction: scores have std ~4 and |max| < ~25,
so exp stays comfortably inside fp32 range and the result is
mathematically identical to jax.nn.softmax.
"""

from contextlib import ExitStack

import numpy as np

import concourse.bacc as bacc
import concourse.tile as tile
from concourse import mybir
from concourse.bass_utils import run_bass_kernel_spmd

F32 = mybir.dt.float32
F32R = mybir.dt.float32r
BF16 = mybir.dt.bfloat16
AFT = mybir.ActivationFunctionType

P = 128      # partitions
S = 2048     # keys (t) per batch
SQ = 1024    # query rows per core
D = 1024     # model dim
NT = S // P  # 16 t-chunks
ND = D // P  # 8 d-chunks
SB = 512     # query-block width in phase C
NSB = SQ // SB
# Warmup: matmuls on uninitialized (zero) SBUF - no DMA dependency, so
# they start at ~6.8us and release the PE HAM clock-gate (1.2->2.4GHz
# ramp), sized to end right as the first y-proj inputs land (~12.5us).
# Zero operands draw almost no multiplier power, so they run at full
# rate; the DVFS activity throttle then trips ~1.5us into the real
# (random-data) matmuls and caps util for ~15us wall.  Measured: a
# full-rate random-data warmup burst (256-wide on m0) pre-triggered the
# throttle but made the recovery LONGER (20.2us vs 14.9us from power
# onset) - net wash - so the warmup stays on zeros and the real work
# just eats the bring-up window at ~35-50% rate.
N_WARMUP_A = 16

B_FULL, S_FULL, D_FULL = 4, 2048, 1024
N_CORES = 8

_NC_CACHE = None
LAST_RESULT = None  # BassKernelResults of the most recent kernel() call
TRACE = False      # set by test.py to capture an NTFF profile
TRACE_DIR = None


def _r(ap):
    return ap.bitcast(F32R)


def _build_nc():
    global _NC_CACHE
    if _NC_CACHE is not None:
        return _NC_CACHE

    nc = bacc.Bacc("TRN2")
    x = nc.declare_dram_parameter("x", [S, D], F32R, isOutput=False)
    xt = nc.declare_dram_parameter("xt", [D, S], F32R, isOutput=False)
    # m pre-gathered on the host into e-major SBUF layout
    # (mg[p, e*1024 + dd*128 + j] = m[dd*128 + p, e*128 + j]): y-proj
    # group e needs m's column block e across all dd row-chunks, and as
    # strided slices of a row-major m those stream at 512-byte packets
    # (measured: half-rate DMA that starved the y projection for ~4us);
    # contiguous per-e blocks move in single [128 x 4KB] full-rate DMAs
    # issued in exact need order.
    mg = nc.declare_dram_parameter("mg", [P, ND * D], F32R, isOutput=False)
    ones = nc.declare_dram_parameter("ones", [P, 2], F32R, isOutput=False)
    out = nc.declare_dram_parameter("out", [SQ, D], F32, isOutput=True)

    with tile.TileContext(nc) as tc, ExitStack() as ctx:
        singles = ctx.enter_context(tc.tile_pool(name="singles", bufs=1))
        ot = singles.tile([P, 2], F32R)

        persist = ctx.enter_context(tc.tile_pool(name="persist", bufs=1))
        # d-chunk d at [:, d*SQ : (d+1)*SQ] (free axis = s)
        yT = persist.tile([P, ND * SQ], F32R, tag="yT")
        # d-chunk d at [:, d*S : (d+1)*S] (free axis = s/t); query rows
        # first (s<1024), key-only rows after.  Persists into phase C as
        # the scores lhsT.
        xT = persist.tile([P, ND * S], F32R, tag="xT")
        # Shared matmul-accumulator pool for the y projection AND the
        # scores groups: keeping one rotation means the first scores
        # group lands in the least-recently-evicted bank instead of
        # waiting on the final y-proj eviction (measured 846ns stall).
        ps_main = ctx.enter_context(
            tc.tile_pool(name="ps_main", bufs=4, space="PSUM"))

        # ---------- phase B: load xt/m, project y ----------
        with tc.tile_pool(name="wA", bufs=1) as wA_pool, \
             tc.tile_pool(name="ps_w", bufs=1, space="PSUM") as ps_w_pool:
            # e-block e at [:, e*D : (e+1)*D]; lhsT for (dd, e) at
            # [:, e*D + dd*P : e*D + (dd+1)*P]
            wt = wA_pool.tile([P, ND * D], F32R, tag="m")

            # Warmup A: clock-ramp matmuls on uninitialized yT contents
            # (yT's first real write is a DVE eviction much later, so no
            # dependency and no delay); garbage results land in a scratch
            # PSUM bank that is never read.
            ps_w = ps_w_pool.tile([P, 512], F32)
            for i in range(N_WARMUP_A):
                nc.tensor.matmul(ps_w[:], yT[:, 0:P], yT[:, SQ:SQ + 512],
                                 start=(i == 0), stop=(i == N_WARMUP_A - 1))

            # DMA order drives arrival order, and through the DVFS window
            # DMA supply and (throttled) PE demand are both marginal, so
            # stream in EXACT need order: m's e=0 block, the xt query
            # stripes, then m's remaining e-blocks.  (Interleaving mg0's
            # dd-chunks with the xt stripes to start the first matmul on
            # 320KB was tried and lost 4us: real DMA packets only start
            # flowing ~8.3us in, the PE went idle >3.4us and re-gated.)
            nc.sync.dma_start(out=wt[:, 0:D], in_=mg[:, 0:D])
            for sh in range(SQ // 512):
                for d in range(ND):
                    nc.sync.dma_start(
                        out=xT[:, d * S + sh * 512: d * S + sh * 512 + 512],
                        in_=xt[d * P:(d + 1) * P, sh * 512: sh * 512 + 512],
                    )
            for e in range(1, ND):
                nc.sync.dma_start(
                    out=wt[:, e * D:(e + 1) * D], in_=mg[:, e * D:(e + 1) * D]
                )
            # xT key-only half (s 1024..2048): needed from scores t=8 on.
            for dd in range(ND):
                nc.sync.dma_start(
                    out=xT[:, dd * S + SQ: dd * S + S],
                    in_=xt[dd * P:(dd + 1) * P, SQ:S],
                )
            # ones (for the sumexp partition-reduce, first used ~84us)
            # issued dead last among phase-B DMAs: at the head its
            # descriptor slot (~0.6us on the Sync queue) would delay mg0.
            nc.sync.dma_start(out=ot[:], in_=ones[:])

            # yT projection; 512-wide stripes only - narrower ones are
            # LDWEIGHTS-bound (190ns load > 107ns mm)
            for e in range(ND):
                for sh in range(SQ // 512):
                    ps = ps_main.tile([P, 512], F32)
                    for dd in range(ND):
                        lhs = wt[:, e * D + dd * P: e * D + (dd + 1) * P]
                        nc.tensor.matmul(
                            ps[:],
                            lhs,
                            xT[:, dd * S + sh * 512: dd * S + sh * 512 + 512],
                            start=(dd == 0), stop=(dd == ND - 1),
                        )
                    nc.vector.tensor_copy(
                        yT[:, e * SQ + sh * 512: e * SQ + sh * 512 + 512],
                        _r(ps[:]),
                    )

        # ---------- phase C: scores -> softmax -> attn @ x ----------
        with tc.tile_pool(name="xc", bufs=1) as xc_pool, \
             tc.tile_pool(name="exp", bufs=1) as exp_pool, \
             tc.tile_pool(name="outp", bufs=4) as out_pool, \
             tc.tile_pool(name="recip", bufs=4) as recip_pool, \
             tc.tile_pool(name="partial", bufs=2) as partial_pool, \
             tc.tile_pool(name="ps_av", bufs=3, space="PSUM") as ps_av, \
             tc.tile_pool(name="ps_sum", bufs=1, space="PSUM") as ps_sum:
            # x natural: t-chunk t at [:, t*D : (t+1)*D]
            xc = xc_pool.tile([P, NT * D], F32R)
            for t in range(NT):
                nc.sync.dma_start(
                    out=xc[:, t * D:(t + 1) * D], in_=x[t * P:(t + 1) * P, :]
                )

            for blk in range(NSB):
                # t-chunk t at [:, t*SB : (t+1)*SB] (free axis = s within blk)
                # (expT must stay f32r: the BIR verifier requires both matmul
                # operands to match when either is f32/f32r, so bf16 expT
                # would force bf16 xc too; and the steady 227ns/mm interval is
                # mm(213) + ~14ns fixed pipeline handoff with LDWEIGHTS fully
                # parallel - faster bf16 weight loads would not shrink it.)
                expT = exp_pool.tile([P, NT * SB], F32R, tag="expT")
                # Softmax denominator: the 16-chunk accumulation runs on
                # the (otherwise idle) DVE as a chain of adds interleaved
                # with the scores loop; the PE then only does one N=2
                # partition-reduce matmul per s-chunk instead of 16
                # LDWEIGHTS-bound ones each (~115ns apiece, ~18us total).
                partial = partial_pool.tile([P, SB], F32R, tag="partial")
                for t in range(NT):
                    ps = ps_main.tile([P, SB], F32)
                    for e in range(ND):
                        nc.tensor.matmul(
                            ps[:],
                            xT[:, e * S + t * P: e * S + (t + 1) * P],
                            yT[:, e * SQ + blk * SB: e * SQ + (blk + 1) * SB],
                            start=(e == 0), stop=(e == ND - 1),
                        )
                    nc.scalar.activation(expT[:, t * SB:(t + 1) * SB], ps[:], AFT.Exp)
                    if t == 1:
                        nc.vector.tensor_add(
                            partial[:], expT[:, 0:SB], expT[:, SB:2 * SB])
                    elif t >= 2:
                        nc.vector.tensor_add(
                            partial[:], partial[:],
                            expT[:, t * SB:(t + 1) * SB])

                for ss in range(SB // P):
                    pss = ps_sum.tile([P, 2], F32)
                    nc.tensor.matmul(
                        pss[:], partial[:, ss * P:(ss + 1) * P], ot[:],
                        start=True, stop=True,
                    )
                    rec = recip_pool.tile([P, 1], F32, tag="rec")
                    nc.vector.reciprocal(rec[:], pss[:, 0:1])

                    for dh in range(2):
                        psa = ps_av.tile([P, 512], F32)
                        for t in range(NT):
                            nc.tensor.matmul(
                                psa[:],
                                expT[:, t * SB + ss * P: t * SB + (ss + 1) * P],
                                xc[:, t * D + dh * 512: t * D + dh * 512 + 512],
                                start=(t == 0), stop=(t == NT - 1),
                            )
                        ob = out_pool.tile([P, 512], F32, tag="ob")
                        row0 = blk * SB + ss * P
                        last = (blk == NSB - 1 and ss == SB // P - 1
                                and dh == 1)
                        if last:
                            # Final block: evict + DMA in two halves, one on
                            # the DVE and one on the ScalarE (Copy with
                            # per-partition scale) so both run concurrently -
                            # this eviction+DMA is the exposed tail after the
                            # last matmul.  The halves land in SEPARATE tiles:
                            # disjoint slices of one tile still serialize on a
                            # tile-level write-write dependency (measured: the
                            # second eviction started 0.6us late either way).
                            ob2 = out_pool.tile([P, 512], F32, tag="ob")
                            nc.scalar.activation(
                                ob2[:, 0:256], psa[:, 256:512], AFT.Copy,
                                scale=rec[:, 0:1])
                            nc.sync.dma_start(
                                out=out[row0:row0 + P,
                                        dh * 512 + 256:dh * 512 + 512],
                                in_=ob2[:, 0:256],
                            )
                            nc.vector.tensor_scalar_mul(
                                ob[:, 0:256], psa[:, 0:256], rec[:, 0:1])
                            nc.sync.dma_start(
                                out=out[row0:row0 + P,
                                        dh * 512:dh * 512 + 256],
                                in_=ob[:, 0:256],
                            )
                        else:
                            nc.vector.tensor_scalar_mul(ob[:], psa[:], rec[:, 0:1])
                            nc.sync.dma_start(
                                out=out[row0:row0 + P, dh * 512:dh * 512 + 512],
                                in_=ob[:],
                            )

    nc.finalize()
    _NC_CACHE = nc
    return nc


def kernel(inputs, Wq, Wk):
    global LAST_RESULT
    x = np.asarray(inputs, dtype=np.float32)
    assert x.shape == (B_FULL, S_FULL, D_FULL)
    # scores = x (Wq Wk^T / 8) x^T: fold the projections and the softmax
    # scale into one host-side sgemm.
    m = (np.asarray(Wq, dtype=np.float32) @ np.asarray(Wk, dtype=np.float32).T) \
        * np.float32(0.125)
    # e-major SBUF layout: mg[p, e*1024 + dd*128 + j] = m[dd*128+p, e*128+j]
    mg = np.ascontiguousarray(
        m.reshape(ND, P, ND, P).transpose(1, 2, 0, 3).reshape(P, ND * D))
    ones = np.ones((P, 2), dtype=np.float32)

    nc = _build_nc()

    in_maps = []
    for c in range(N_CORES):
        b, h = c // 2, c % 2
        xb = x[b]
        if h:
            xb = np.concatenate([xb[SQ:], xb[:SQ]], axis=0)
        in_maps.append({
            "x": np.ascontiguousarray(xb),
            "xt": np.ascontiguousarray(xb.T),
            "mg": mg,
            "ones": ones,
        })

    kwargs = {}
    if TRACE:
        kwargs = {"trace": True, "tmpdir": TRACE_DIR}
    res = run_bass_kernel_spmd(nc, in_maps, list(range(N_CORES)), **kwargs)
    LAST_RESULT = res

    full = np.empty((B_FULL, S_FULL, D_FULL), dtype=np.float32)
    for c in range(N_CORES):
        b, h = c // 2, c % 2
        full[b, h * SQ:(h + 1) * SQ, :] = res.results[c]["out"]
    return full
